# revision 4
# baseline (speedup 1.0000x reference)
"""Trainium2 Bass kernel for CentroidEdgeConvNet (2-layer mean-aggregation GNN).

Reference computation (N=100000 nodes, DEG=16, F=H=128, C=40):
    h1 = relu(mean_k feats[nbr[i,k]] @ W0 + b0)            # [N, H]
    out = log_softmax(mean_k h1[nbr2[i,k]] @ W1 + b1)      # [N, C],  nbr2 = neighbors[ids]

Sharding: nodes data-parallel over 8 cores (12500/core, padded to 12544 = 98
tiles of 128).  feats + weights replicated.  Phase 1 computes the local h1
shard via indirect-DMA gather (512B rows) from HBM; an AllGather exchanges h1
shards; phase 2 gathers from the full h1 table and finishes with W1 +
log_softmax.  The 1/16 neighbor-mean scaling is folded into W0/W1 on the host.

Host-side index prep (int32 gather index arrays, one per core) keeps the
device program identical across cores (pure SPMD + one collective).
"""

import numpy as np

import concourse.bacc as bacc
import concourse.bass as bass
import concourse.mybir as mybir
import concourse.tile as tile
from concourse.bass import IndirectOffsetOnAxis
from concourse.bass_utils import run_bass_kernel_spmd
from concourse.masks import make_identity

# Problem constants (hardcoded per harness contract)
N_NODES = 100000
DEG = 16
F = 128
H = 128
C = 40
NCORES = 8
P = 128

NSHARD = N_NODES // NCORES          # 12500
TILES = (NSHARD + P - 1) // P       # 98
NP_ROWS = TILES * P                 # 12544 padded shard rows
TBL_ROWS = NP_ROWS * NCORES         # 100352 rows in the all-gathered h1 table

F32 = mybir.dt.float32
I32 = mybir.dt.int32


def build_program(n_nodes=N_NODES, tiles=TILES, ncores=NCORES, ag_stripes=1):
    """Build the SPMD Bass program (identical on all cores)."""
    np_rows = tiles * P
    tbl_rows = np_rows * ncores
    assert tiles % ag_stripes == 0
    stripe_tiles = tiles // ag_stripes

    nc = bacc.Bacc(
        "TRN2", target_bir_lowering=False, debug=False, num_devices=ncores
    )

    feats_t = nc.dram_tensor("feats", [n_nodes, F], F32, kind="ExternalInput")
    w0_t = nc.dram_tensor("w0", [F, H], F32, kind="ExternalInput")
    b0_t = nc.dram_tensor("b0", [H, 1], F32, kind="ExternalInput")
    w1_t = nc.dram_tensor("w1", [H, C], F32, kind="ExternalInput")
    b1_t = nc.dram_tensor("b1", [C, 1], F32, kind="ExternalInput")
    idx1_t = nc.dram_tensor("idx1", [P, tiles * DEG], I32, kind="ExternalInput")
    idx2_t = nc.dram_tensor("idx2", [P, tiles * DEG], I32, kind="ExternalInput")
    out_t = nc.dram_tensor("out", [P, tiles * C], F32, kind="ExternalOutput")

    AF = mybir.ActivationFunctionType
    ALU = mybir.AluOpType

    with tile.TileContext(nc) as tc:
        with (
            tc.tile_pool(name="const", bufs=1) as cpool,
            tc.tile_pool(name="gath", bufs=4) as gpool,
            tc.tile_pool(name="work", bufs=3) as wpool,
            tc.tile_pool(name="small", bufs=3) as spool,
            tc.tile_pool(name="outp", bufs=1) as opool,
            tc.tile_pool(name="ps", bufs=2, space="PSUM") as pspool,
            tc.tile_pool(name="dram", bufs=1, space="DRAM") as dpool,
        ):
            # --- constants / parameters ---
            w0_sb = cpool.tile([F, H], F32, name="w0_sb")
            nc.sync.dma_start(w0_sb[:], w0_t.ap())
            w1_sb = cpool.tile([H, C], F32, name="w1_sb")
            nc.sync.dma_start(w1_sb[:], w1_t.ap())
            b0_sb = cpool.tile([H, 1], F32, name="b0_sb")
            nc.sync.dma_start(b0_sb[:], b0_t.ap())
            b1_sb = cpool.tile([C, 1], F32, name="b1_sb")
            nc.sync.dma_start(b1_sb[:], b1_t.ap())
            ident = cpool.tile([P, P], F32, name="ident")
            make_identity(nc, ident[:])

            idx1_sb = cpool.tile([P, tiles * DEG], I32, name="idx1_sb")
            nc.sync.dma_start(idx1_sb[:], idx1_t.ap())
            idx2_sb = cpool.tile([P, tiles * DEG], I32, name="idx2_sb")
            nc.sync.dma_start(idx2_sb[:], idx2_t.ap())

            out_acc = opool.tile([P, tiles * C], F32, name="out_acc")

            h1_shard = dpool.tile([np_rows, F], F32, name="h1_shard")
            h1_full = dpool.tile(
                [tbl_rows, F], F32, name="h1_full", addr_space="Shared"
            )

            # --- phase 1: local h1 shard ---
            for t in range(tiles):
                g = gpool.tile([P, DEG, F], F32, name="g", tag="g")
                # HW DynamicAP indirect DMA only honors one offset per
                # partition -> 16 gathers of 128 rows each per node-tile
                for k in range(DEG):
                    nc.gpsimd.indirect_dma_start(
                        out=g[:, k, :],
                        out_offset=None,
                        in_=feats_t.ap(),
                        in_offset=IndirectOffsetOnAxis(
                            ap=idx1_sb[:, t * DEG + k : t * DEG + k + 1], axis=0
                        ),
                    )
                # sum over the 16 gathered neighbor rows (1/16 folded into W0)
                m1 = wpool.tile([P, F], F32, name="m1", tag="m")
                nc.vector.tensor_reduce(
                    out=m1[:],
                    in_=g.rearrange("p a b -> p b a"),
                    axis=mybir.AxisListType.X,
                    op=ALU.add,
                )
                m1t_p = pspool.tile([P, P], F32, name="m1t_p", tag="mt_p")
                nc.tensor.transpose(m1t_p[:], m1[:], ident[:])
                m1t = wpool.tile([P, P], F32, name="m1t", tag="mt")
                nc.scalar.copy(m1t[:], m1t_p[:])
                h1t_p = pspool.tile([H, P], F32, name="h1t_p", tag="mm_p")
                nc.tensor.matmul(
                    h1t_p[:], lhsT=w0_sb[:], rhs=m1t[:], start=True, stop=True
                )
                h1t = wpool.tile([H, P], F32, name="h1t", tag="h1t")
                nc.scalar.activation(h1t[:], h1t_p[:], AF.Relu, bias=b0_sb[:, 0:1])
                h1_p = pspool.tile([P, H], F32, name="h1_p", tag="t2_p")
                nc.tensor.transpose(h1_p[:], h1t[:], ident[:])
                h1s = wpool.tile([P, H], F32, name="h1s", tag="h1s")
                nc.vector.tensor_copy(h1s[:], h1_p[:])
                nc.sync.dma_start(h1_shard[t * P : (t + 1) * P, :], h1s[:])

            # --- exchange h1 shards (optionally striped for overlap) ---
            srows = stripe_tiles * P
            for s in range(ag_stripes):
                nc.gpsimd.collective_compute(
                    "AllGather",
                    ALU.bypass,
                    replica_groups=[list(range(ncores))],
                    ins=[h1_shard[s * srows : (s + 1) * srows, :].opt()],
                    outs=[
                        h1_full[
                            s * srows * ncores : (s + 1) * srows * ncores, :
                        ].opt()
                    ],
                )

            # --- phase 2: gather h1, W1, log_softmax ---
            for t in range(tiles):
                g2 = gpool.tile([P, DEG, F], F32, name="g2", tag="g")
                for k in range(DEG):
                    nc.gpsimd.indirect_dma_start(
                        out=g2[:, k, :],
                        out_offset=None,
                        in_=h1_full[:],
                        in_offset=IndirectOffsetOnAxis(
                            ap=idx2_sb[:, t * DEG + k : t * DEG + k + 1], axis=0
                        ),
                    )
                m2 = wpool.tile([P, H], F32, name="m2", tag="m")
                nc.vector.tensor_reduce(
                    out=m2[:],
                    in_=g2.rearrange("p a b -> p b a"),
                    axis=mybir.AxisListType.X,
                    op=ALU.add,
                )
                m2t_p = pspool.tile([P, P], F32, name="m2t_p", tag="mt_p")
                nc.tensor.transpose(m2t_p[:], m2[:], ident[:])
                m2t = wpool.tile([P, P], F32, name="m2t", tag="mt")
                nc.scalar.copy(m2t[:], m2t_p[:])
                lg_p = pspool.tile([C, P], F32, name="lg_p", tag="mm_p")
                nc.tensor.matmul(
                    lg_p[:], lhsT=w1_sb[:], rhs=m2t[:], start=True, stop=True
                )
                lg = wpool.tile([C, P], F32, name="lg", tag="lg")
                nc.scalar.activation(lg[:], lg_p[:], AF.Identity, bias=b1_sb[:, 0:1])
                lgt_p = pspool.tile([P, C], F32, name="lgt_p", tag="t2_p")
                nc.tensor.transpose(lgt_p[:], lg[:], ident[:C, :C])
                # log_softmax over the C free elements per node-partition
                nmax = spool.tile([P, 1], F32, name="nmax", tag="nmax")
                nc.vector.tensor_reduce(
                    out=nmax[:],
                    in_=lgt_p[:],
                    axis=mybir.AxisListType.X,
                    op=ALU.max,
                    negate=True,
                )
                e = wpool.tile([P, C], F32, name="e", tag="e")
                ssum = spool.tile([P, 1], F32, name="ssum", tag="ssum")
                nc.scalar.activation(
                    e[:], lgt_p[:], AF.Exp, bias=nmax[:, 0:1], accum_out=ssum[:, 0:1]
                )
                lse = spool.tile([P, 1], F32, name="lse", tag="lse")
                nc.scalar.activation(lse[:], ssum[:], AF.Ln)
                nml = spool.tile([P, 1], F32, name="nml", tag="nml")
                nc.vector.tensor_sub(nml[:], nmax[:], lse[:])
                nc.scalar.activation(
                    out_acc[:, t * C : (t + 1) * C],
                    lgt_p[:],
                    AF.Identity,
                    bias=nml[:, 0:1],
                )

            nc.sync.dma_start(out_t.ap(), out_acc[:])

    nc.compile()
    return nc


def make_host_inputs(feats, W0, b0, W1, b1, ids, neighbors, n_nodes=N_NODES,
                     tiles=TILES, ncores=NCORES, ag_stripes=1):
    """Build per-core input maps (index prep + weight folding on host)."""
    np_rows = tiles * P
    nshard = n_nodes // ncores
    assert tiles % ag_stripes == 0
    stripe_rows = (tiles // ag_stripes) * P

    neighbors = np.asarray(neighbors).astype(np.int64)
    ids = np.asarray(ids).astype(np.int64)
    nbr2 = neighbors[ids]  # [n_out, DEG] layer-2 neighbor sets

    feats = np.ascontiguousarray(np.asarray(feats, np.float32))
    w0s = np.ascontiguousarray(np.asarray(W0, np.float32) / DEG)
    w1s = np.ascontiguousarray(np.asarray(W1, np.float32) / DEG)
    b0c = np.ascontiguousarray(np.asarray(b0, np.float32).reshape(H, 1))
    b1c = np.ascontiguousarray(np.asarray(b1, np.float32).reshape(C, 1))

    # map node id -> row in the all-gathered (padded, possibly striped) table
    def table_row(j):
        owner = j // nshard
        local = j - owner * nshard
        stripe = local // stripe_rows
        within = local - stripe * stripe_rows
        return (stripe * ncores + owner) * stripe_rows + within

    in_maps = []
    for c in range(ncores):
        base = c * nshard
        # phase-1 node ids for this core's padded shard, clamped for pad slots
        node = np.minimum(base + np.arange(np_rows), n_nodes - 1)
        i1 = neighbors[node]                       # [np_rows, DEG]
        i1 = i1.reshape(tiles, P, DEG).transpose(1, 0, 2).reshape(P, tiles * DEG)
        out_row = np.minimum(base + np.arange(np_rows), n_nodes - 1)
        i2 = table_row(nbr2[out_row])              # [np_rows, DEG]
        i2 = i2.reshape(tiles, P, DEG).transpose(1, 0, 2).reshape(P, tiles * DEG)
        in_maps.append(
            {
                "feats": feats,
                "w0": w0s,
                "b0": b0c,
                "w1": w1s,
                "b1": b1c,
                "idx1": np.ascontiguousarray(i1.astype(np.int32)),
                "idx2": np.ascontiguousarray(i2.astype(np.int32)),
            }
        )
    return in_maps


def unshard_output(results, n_nodes=N_NODES, tiles=TILES, ncores=NCORES):
    """results: list of per-core {"out": [P, tiles*C]} -> full [n_nodes, C]."""
    nshard = n_nodes // ncores
    parts = []
    for c in range(ncores):
        o = np.asarray(results[c]["out"]).reshape(P, tiles, C)
        o = o.transpose(1, 0, 2).reshape(tiles * P, C)[:nshard]
        parts.append(o)
    return np.ascontiguousarray(np.concatenate(parts, axis=0).astype(np.float32))


_NC_CACHE = {}


def _get_program(key=(N_NODES, TILES, NCORES, 1)):
    if key not in _NC_CACHE:
        _NC_CACHE[key] = build_program(*key)
    return _NC_CACHE[key]


def kernel(**inputs):
    nc = _get_program()
    in_maps = make_host_inputs(
        inputs["feats"], inputs["W0"], inputs["b0"], inputs["W1"], inputs["b1"],
        inputs["ids"], inputs["neighbors"],
    )
    res = run_bass_kernel_spmd(nc, in_maps, core_ids=list(range(NCORES)))
    return unshard_output(res.results)
